# revision 1
# baseline (speedup 1.0000x reference)
"""MoE block (B=4, T=1024, D=1024, H=4096, E=8, top-2) on 8 Trainium2 cores.

Strategy: expert-parallel. The reference computes every expert for every
token and then keeps only the top-2; the output only depends on the top-2
selections, so we route: host computes the (tiny) gating in float64, core e
runs the dense FFN for expert e over just the tokens routed to it
(transposed layout, zero on-device transposes), and the host combines the
two selected expert outputs per token with the gate weights.

Device kernel per core (C = max tokens routed to any expert, padded):
    h1T[H, C]  = relu(w1[e].T @ xT + b1)   (bf16 in, fp32 accum)
    yT [D, C]  = w2[e].T @ h1T + b2
All matmuls run in bf16 at the full PE rate; PE-roofline is ~C*12288 cycles.
"""

import numpy as np
import ml_dtypes

B, T, D, H, E = 4, 1024, 1024, 4096, 8
TOP_K = 2
N_CORES = 8
KD = D // 128   # 8  K-tiles over D
KH = H // 128   # 32 K-tiles over H

_PROGRAM_CACHE: dict = {}


def _build_program(C: int):
    """Bass/Tile program: dense FFN for one expert over C token columns."""
    if C in _PROGRAM_CACHE:
        return _PROGRAM_CACHE[C]

    from contextlib import ExitStack
    import concourse.bacc as bacc
    import concourse.mybir as mybir
    import concourse.tile as tile

    bf = mybir.dt.bfloat16
    f32 = mybir.dt.float32
    AFT = mybir.ActivationFunctionType

    nc = bacc.Bacc("TRN2", target_bir_lowering=False, debug=False,
                   num_devices=N_CORES)

    xt_d = nc.dram_tensor("xt", [D, C], bf, kind="ExternalInput")
    w1_d = nc.dram_tensor("w1q", [KH, 128, D], bf, kind="ExternalInput")
    w2_d = nc.dram_tensor("w2q", [KD, 128, H], bf, kind="ExternalInput")
    b1_d = nc.dram_tensor("b1t", [128, KH], f32, kind="ExternalInput")
    b2_d = nc.dram_tensor("b2t", [128, KD], f32, kind="ExternalInput")
    y_d = nc.dram_tensor("y", [D, C], f32, kind="ExternalOutput")

    chunks = [(s, min(512, C - s)) for s in range(0, C, 512)]

    with tile.TileContext(nc) as tc, ExitStack() as ctx:
        pers = ctx.enter_context(tc.tile_pool(name="pers", bufs=1))
        wpool = ctx.enter_context(tc.tile_pool(name="w", bufs=2))
        opool = ctx.enter_context(tc.tile_pool(name="o", bufs=2))
        psum = ctx.enter_context(tc.tile_pool(name="ps", bufs=2, space="PSUM"))

        xt = pers.tile([128, KD, C], bf)
        for k in range(KD):
            nc.sync.dma_start(xt[:, k, :], xt_d[k * 128:(k + 1) * 128, :])
        b1s = pers.tile([128, KH], f32)
        nc.sync.dma_start(b1s[:], b1_d[:])
        b2s = pers.tile([128, KD], f32)
        nc.sync.dma_start(b2s[:], b2_d[:])
        h1 = pers.tile([128, KH, C], bf)

        # Phase A: h1T[m] = relu(w1.T @ xT + b1), m over H tiles
        for m in range(KH):
            w1s = wpool.tile([128, D], bf, tag="w1s")
            nc.sync.dma_start(w1s[:], w1_d[m])
            acc = psum.tile([128, C], f32, tag="acc")
            for k in range(KD):
                for (s, w) in chunks:
                    nc.tensor.matmul(acc[:, s:s + w],
                                     w1s[:, k * 128:(k + 1) * 128],
                                     xt[:, k, s:s + w],
                                     start=(k == 0), stop=(k == KD - 1))
            nc.scalar.activation(h1[:, m, :], acc[:], AFT.Relu,
                                 bias=b1s[:, m:m + 1])

        # Phase B: yT[d] = w2.T @ h1T + b2, d over D tiles
        for d in range(KD):
            w2s = wpool.tile([128, H], bf, tag="w2s")
            nc.sync.dma_start(w2s[:], w2_d[d])
            acc = psum.tile([128, C], f32, tag="acc")
            for k in range(KH):
                for (s, w) in chunks:
                    nc.tensor.matmul(acc[:, s:s + w],
                                     w2s[:, k * 128:(k + 1) * 128],
                                     h1[:, k, s:s + w],
                                     start=(k == 0), stop=(k == KH - 1))
            ost = opool.tile([128, C], f32, tag="ost")
            nc.scalar.activation(ost[:], acc[:], AFT.Identity,
                                 bias=b2s[:, d:d + 1])
            nc.sync.dma_start(y_d[d * 128:(d + 1) * 128, :], ost[:])

    nc.compile()
    _PROGRAM_CACHE[C] = nc
    return nc


def _route(x2, gate_w, gate_b):
    """Float64 gating: returns (top_idx [N,2], top_gate [N,2])."""
    logits = x2.astype(np.float64) @ gate_w.astype(np.float64) \
        + gate_b.astype(np.float64)
    z = np.exp(logits - logits.max(axis=1, keepdims=True))
    probs = z / z.sum(axis=1, keepdims=True)
    top = np.argsort(-logits, axis=1, kind="stable")[:, :TOP_K]
    gv = np.take_along_axis(probs, top, axis=1)
    return top, gv


def _pack_weights(w1e, w2e, b1e, b2e):
    bfl = ml_dtypes.bfloat16
    w1q = np.ascontiguousarray(
        w1e.reshape(KD, 128, KH, 128).transpose(2, 1, 0, 3)
        .reshape(KH, 128, D).astype(bfl))
    w2q = np.ascontiguousarray(
        w2e.reshape(KH, 128, KD, 128).transpose(2, 1, 0, 3)
        .reshape(KD, 128, H).astype(bfl))
    b1t = np.ascontiguousarray(b1e.reshape(KH, 128).T.astype(np.float32))
    b2t = np.ascontiguousarray(b2e.reshape(KD, 128).T.astype(np.float32))
    return w1q, w2q, b1t, b2t


def kernel(x, gate_w, gate_b, w1, b1, w2, b2, _bench_hook=None):
    from concourse.bass_utils import run_bass_kernel_spmd

    bfl = ml_dtypes.bfloat16
    x = np.asarray(x, np.float32)
    x2 = x.reshape(-1, D)                       # [N, D], N = B*T
    N = x2.shape[0]

    top, gv = _route(x2, np.asarray(gate_w), np.asarray(gate_b))

    tok_lists = [np.where((top == e).any(axis=1))[0] for e in range(E)]
    maxload = max(1, max(len(t) for t in tok_lists))
    C = (maxload + 7) // 8 * 8

    nc = _build_program(C)

    x2b = x2.astype(bfl)
    in_maps = []
    for e in range(E):
        toks = tok_lists[e]
        xt = np.zeros((D, C), bfl)
        if len(toks):
            xt[:, :len(toks)] = x2b[toks].T
        w1q, w2q, b1t, b2t = _pack_weights(
            np.asarray(w1[e], np.float32), np.asarray(w2[e], np.float32),
            np.asarray(b1[e], np.float32), np.asarray(b2[e], np.float32))
        in_maps.append({"xt": xt, "w1q": w1q, "w2q": w2q,
                        "b1t": b1t, "b2t": b2t})

    res = run_bass_kernel_spmd(nc, in_maps, core_ids=list(range(N_CORES)))
    if _bench_hook is not None:
        _bench_hook(nc, in_maps)

    out = np.zeros((N, D), np.float64)
    for e in range(E):
        toks = tok_lists[e]
        if not len(toks):
            continue
        ye = res.results[e]["y"]                # [D, C] fp32
        ge = np.where(top[toks] == e, gv[toks], 0.0).sum(axis=1)
        out[toks] += ge[:, None] * ye[:, :len(toks)].T.astype(np.float64)

    return out.astype(np.float32).reshape(B, T, D)
